# revision 1
# baseline (speedup 1.0000x reference)
import numpy as np
import jax
import jax.numpy as jnp
from jax import lax

# CapsuleNet forward, data-parallel over the 8 NeuronCores.
# Full batch N=512 is sharded 8 x 64 along the batch dim; all parameters
# (conv kernels, capsule weights, routing prior) are replicated. Routing is
# fully batch-local so the forward needs no cross-device communication.

N_CORES = 8


def _squash(vec):
    # vec: [..., caps, dim]; squash along last dim
    lensq = jnp.sum(vec * vec, axis=2)
    length = jnp.sqrt(lensq)
    scale = (lensq / (1.0 + lensq) / length)[:, :, None]
    return vec * scale


def _forward(x, conv1_w, conv1_b, conv2_w, conv2_b, caps_w, b_prior):
    n_iterations = 3
    output_caps, output_dim = 10, 16
    dn = ("NCHW", "OIHW", "NCHW")

    # conv1 + relu: [n,1,28,28] -> [n,256,20,20]
    h = lax.conv_general_dilated(x, conv1_w, (1, 1), "VALID", dimension_numbers=dn)
    h = jax.nn.relu(h + conv1_b[None, :, None, None])

    # primary caps conv: k=9 s=2 -> [n,256,6,6]
    h = lax.conv_general_dilated(h, conv2_w, (2, 2), "VALID", dimension_numbers=dn)
    h = h + conv2_b[None, :, None, None]
    n, C, H, W = h.shape
    u = h.reshape(n, 32, 8, H, W).transpose(0, 1, 3, 4, 2).reshape(n, 32 * H * W, 8)
    u = _squash(u)  # [n, 1152, 8]

    # prediction vectors: u_hat [n, 1152, 10, 16]
    u_hat = jnp.einsum("bid,ido->bio", u, caps_w).reshape(
        n, 1152, output_caps, output_dim
    )

    # dynamic routing-by-agreement
    c = jax.nn.softmax(b_prior, axis=1)  # [1152, 10]
    s = jnp.sum(c[None, :, :, None] * u_hat, axis=1)  # [n, 10, 16]
    v = _squash(s)
    b_batch = jnp.broadcast_to(b_prior, (n, 1152, output_caps))
    for _ in range(n_iterations):
        b_batch = b_batch + jnp.sum(u_hat * v[:, None, :, :], axis=-1)
        c = jax.nn.softmax(b_batch, axis=2)[..., None]
        s = jnp.sum(c * u_hat, axis=1)
        v = _squash(s)

    probs = jnp.sqrt(jnp.sum(v * v, axis=2))  # [n, 10]
    return v, probs


_pmapped = jax.pmap(
    _forward,
    in_axes=(0, None, None, None, None, None, None),
    out_axes=0,
)


def kernel(x, conv1_w, conv1_b, conv2_w, conv2_b, caps_w, b_prior):
    N = x.shape[0]
    per = N // N_CORES
    xs = np.ascontiguousarray(x.reshape(N_CORES, per, *x.shape[1:]))
    v, probs = _pmapped(xs, conv1_w, conv1_b, conv2_w, conv2_b, caps_w, b_prior)
    v = np.asarray(v).reshape(N, 10, 16).astype(np.float32)
    probs = np.asarray(probs).reshape(N, 10).astype(np.float32)
    return v, probs
